# revision 54
# baseline (speedup 1.0000x reference)
"""Trainium2 Bass kernel for causal multi-head attention (dense transformer block).

Problem: x[2,2048,1024] -> qkv proj -> 16-head causal attention (scale 1/sqrt(1024))
         -> out proj.  8 NeuronCores.

Sharding: core c handles batch b=c//4 and head-group r=c%4 (heads 4r..4r+3).
  - qkv weights column-sharded by head group (q/k/v slices of 256 cols each)
  - attention computed fully on-core in a transposed layout:
      S^T[k,q] = K^T-chunk (stationary) x Q^T (moving) on the PE
      P = exp(S/32) with causal masking; denominator obtained by appending a
      ones-column to V so that O^T = [V|1]^T P gives sums in the last row.
  - AllGather (bf16, groups of 4 cores sharing a batch) assembles all heads'
    outputs feature-major; out-proj is column-sharded with an all-gathered
    feature dim; biases are applied via rank-1 (K=1) matmul accumulation.

kernel(**inputs) takes the FULL fp32 inputs and returns the FULL output.
"""

import sys

sys.path.insert(0, "/opt/trn_rl_repo")

import numpy as np

import concourse.bass as bass
import concourse.bacc as bacc
import concourse.mybir as mybir
import concourse.tile as tile
from concourse.bass import ds, ts
from concourse.bass_utils import run_bass_kernel_spmd
from concourse.masks import make_upper_triangular

F32 = mybir.dt.float32
BF16 = mybir.dt.bfloat16

# ---------------------------------------------------------------- dims
BS, L, DM, H = 2, 2048, 1024, 16
HD = 64                      # head dim
NCORES = 8
GRP = 4                      # cores per batch group (head-parallel)
HLOC = H // GRP              # heads per core = 4
FLOC = HLOC * HD             # local features = 256
SCALE = 1.0 / float(np.sqrt(DM))
REPLICA_GROUPS = [[0, 1, 2, 3], [4, 5, 6, 7]]


class Cfg:
    """Geometry (parametrized so a small config can be tested quickly)."""

    def __init__(self, L=L, DM=DM, hloc=HLOC, hd=HD, npass=2):
        self.L, self.DM, self.HLOC, self.HD, self.NPASS = L, DM, hloc, hd, npass
        self.FLOC = hloc * hd
        self.NT = L // 128           # 128-token tiles
        self.NB = L // 512           # 512-token blocks
        self.NDM = DM // 128         # dmodel chunks
        self.PW = L // npass         # pass width (q columns per pass)
        self.NSUP = self.PW // 512   # 512-q supers per pass
        self.NFT = self.FLOC // 128  # feature tiles for Q^T/K^T (2)
        self.scale = 1.0 / float(np.sqrt(DM))
        assert self.PW % 512 == 0 and self.FLOC % 128 == 0


def build_body(nc, cfg, x, wqkv, bq, bk, bv, wo, bo, out, groups):
    """Emit the per-core program (Tile framework) for one iteration."""
    NT, NB, NDM, PW, NSUP, NFT = cfg.NT, cfg.NB, cfg.NDM, cfg.PW, cfg.NSUP, cfg.NFT
    HLOCc, HDc, FLOCc = cfg.HLOC, cfg.HD, cfg.FLOC
    Lc, DMc = cfg.L, cfg.DM
    tc = nc.tc

    with tc.tile_pool(name="const", bufs=1) as constp, \
         tc.tile_pool(name="persist", bufs=1) as pp:
        # ---------------- persistent SBUF tensors
        xT = pp.tile([128, NDM, Lc], BF16)                 # x^T  (dm-major)
        wqkvb = pp.tile([128, NDM, 3 * FLOCc], BF16)       # [wq|wk|wv] packed
        wqb = wqkvb[:, :, 0:FLOCc]
        wkb = wqkvb[:, :, FLOCc : 2 * FLOCc]
        wvb = wqkvb[:, :, 2 * FLOCc : 3 * FLOCc]
        wob = pp.tile([128, NDM, FLOCc], BF16)
        QT = pp.tile([128, NFT, Lc], BF16)                 # Q^T feature-major
        KT = pp.tile([128, NFT, Lc], BF16)
        Vb = pp.tile([128, NT, HLOCc * (HDc + 1)], BF16)   # [V | ones] per token tile
        OTs = pp.tile([128, NFT, Lc], BF16)                # attention out^T (feature-major)

        # ---------------- single PSUM pool for the whole kernel
        # bank budget: stile [128,1024]x2 = 4, otile [65,512]x2 = 2,
        # work [128,512]x2 = 2  ->  8 banks. One pool, opened before staging,
        # so no pool-boundary barrier ever lands on the critical path.
        psum_cm = tc.tile_pool(name="psum", bufs=2, space="PSUM")
        psum = psum_cm.__enter__()

        # PE warmup: junk matmuls at the head so the p-state ramp happens on
        # dead time (the DMA-bound front), not on the first real matmuls.
        # Scratch lives in the persist pool: a dedicated pool's release would
        # serialize the staging pool's allocation behind the warmup.
        NWARM = 70
        wsrc_t = pp.tile([128, 512], BF16, name="wsrc_t")
        nc.vector.memset(wsrc_t, 0.25)
        wps = psum.tile([128, 1024], F32, tag="stile", name="wps")
        for r in range(NWARM):
            nc.tensor.matmul(wps[:, 0:512], wsrc_t[:, 0:128], wsrc_t,
                             start=(r == 0), stop=(r == NWARM - 1))
        wout_t = pp.tile([128, 512], F32, name="wout_t")
        nc.vector.tensor_copy(wout_t, wps[:, 0:512])

        # ---------------- constants (emitted off the Pool DMA path where possible)
        trimask = constp.tile([128, 128], BF16)
        ones_r = constp.tile([1, 128], BF16)
        bq_f = constp.tile([128, NFT], F32)
        bk_f = constp.tile([128, NFT], F32)
        bvb = constp.tile([1, FLOCc], BF16)
        bob = constp.tile([1, FLOCc], BF16)

        def emit_consts():
            make_upper_triangular(nc, trimask, val=1.0, diag=True)
            nc.vector.memset(ones_r, 1.0)
            # biases go over HWDGE (f32) + tiny DVE casts -- keeps the serial
            # Pool SWDGE queue free for the big weight/x cast-loads
            nc.sync.dma_start(bq_f, bq.rearrange("(f p) -> p f", p=128))
            nc.sync.dma_start(bk_f, bk.rearrange("(f p) -> p f", p=128))
            bv_st = constp.tile([1, 2 * FLOCc], F32, name="bv_st")
            nc.sync.dma_start(bv_st[:, 0:FLOCc], bv.rearrange("(a b) -> a b", a=1))
            nc.sync.dma_start(bv_st[:, FLOCc : 2 * FLOCc], bo.rearrange("(a b) -> a b", a=1))
            nc.vector.tensor_copy(bvb, bv_st[:, 0:FLOCc])
            nc.vector.tensor_copy(bob, bv_st[:, FLOCc : 2 * FLOCc])
            # ones columns of Vb
            nc.vector.memset(
                Vb.rearrange("p t (h u) -> p t h u", u=HDc + 1)[:, :, :, HDc : HDc + 1], 1.0
            )

        # ---------------- weight + x staging
        # A single 3D-output DMA-transpose of a [128, DM] tile yields the
        # natural chunking xT[p, c, tok] = x^T[128*c + p, tok]; weight loads
        # use the same chunking so contractions line up.
        # All casts fp32->bf16 happen inside gpsimd (SWDGE) DMAs.
        # Emission order = DMA queue order: first x tiles feed the transposes
        # that gate the first qkv matmuls; weights trickle in behind them.
        with tc.tile_pool(name="stage", bufs=2) as sp:
            # The Pool SWDGE path (casts during DMA, serial per-DMA, queue
            # depth 4) carries few BIG transfers, ordered by first use:
            # x-block0, wq, wk, wv, x-block1..3, wo.  Each x cast-DMA covers 4
            # token tiles; transposes fan out per 128-token tile on HWDGE.
            xv = x.rearrange("(b t p) dm -> b p t dm", p=128, t=4)

            def stage_xblock(b4):
                xbf4 = sp.tile([128, 4, DMc], BF16, tag="xbf", name="xbf4")
                nc.gpsimd.dma_start(xbf4, xv[b4])
                for t4 in range(4):
                    nc.sync.dma_start(
                        xT[:, :, ts(4 * b4 + t4, 128)], xbf4[:, t4, :], transpose=True
                    )

            stage_xblock(0)
            stage_xblock(1)
            nc.gpsimd.dma_start(wqkvb, wqkv.rearrange("(c p) f -> p c f", p=128))
            emit_consts()
            for b4 in range(2, NT // 4):
                stage_xblock(b4)
            nc.gpsimd.dma_start(wob, wo.rearrange("(c p) f -> p c f", p=128))

        # ---------------- qkv projection
        if True:
            psqk = psum
            psv_p = psum
            for tb in range(NB):
                for ft in range(NFT):
                    qk = psqk.tile([128, 1024], F32, tag="stile", name="qk")
                    for c in range(NDM):
                        nc.tensor.matmul(
                            qk[:, 0:512], wqb[:, c, ts(ft, 128)], xT[:, c, ts(tb, 512)],
                            start=(c == 0), stop=(c == NDM - 1),
                        )
                    nc.scalar.activation(QT[:, ft, ts(tb, 512)], qk[:, 0:512],
                                         mybir.ActivationFunctionType.Identity,
                                         bias=bq_f[:, ft : ft + 1])
                    for c in range(NDM):
                        nc.tensor.matmul(
                            qk[:, 512:1024], wkb[:, c, ts(ft, 128)], xT[:, c, ts(tb, 512)],
                            start=(c == 0), stop=(c == NDM - 1),
                        )
                    nc.scalar.activation(KT[:, ft, ts(tb, 512)], qk[:, 512:1024],
                                         mybir.ActivationFunctionType.Identity,
                                         bias=bk_f[:, ft : ft + 1])
                for tt in range(tb * 4, tb * 4 + 4):
                    psv_full = psv_p.tile([128, 512], F32, tag="work", name="psv_full")
                    psv = psv_full[:, 0:FLOCc]
                    for c in range(NDM):
                        nc.tensor.matmul(
                            psv, xT[:, c, ts(tt, 128)], wvb[:, c, :],
                            start=(c == 0), stop=False,
                        )
                    nc.tensor.matmul(psv, ones_r, bvb, start=False, stop=True)
                    nc.scalar.copy(
                        Vb[:, tt, :].rearrange("p (h u) -> p h u", u=HDc + 1)[:, :, 0:HDc],
                        psv.rearrange("p (h d) -> p h d", d=HDc),
                    )

        # ---------------- attention + allgather + out projection
        pss = psum
        psop = psum
        psout = psum
        with tc.tile_pool(name="pbuf", bufs=6) as pbp, \
             tc.tile_pool(name="nrm", bufs=6) as nrm, \
             tc.tile_pool(name="of", bufs=3) as ofp, \
             tc.tile_pool(name="osb", bufs=3) as osbp, \
             tc.tile_pool(name="dram", bufs=2, space="DRAM") as dramp:
            pending_agproj = {False: lambda pp: emit_agproj(pp, 0),
                              True: lambda pp: emit_agproj(pp, 1)}
            for p in range(cfg.NPASS):
                ilast = (p + 1) * PW // 128 - 1

                def emit_scores(h, i):
                    hf, hp = h // 2, h % 2
                    S = pss.tile([128, PW], F32, tag="stile", name="S")
                    for j2 in range(NSUP):
                        qs = p * PW + 512 * j2
                        if 128 * i < qs + 512:
                            # causal: columns below the diagonal are never read
                            al = max(0, 128 * i - qs)
                            nc.tensor.matmul(
                                S[:, ds(512 * j2 + al, 512 - al)],
                                KT[64 * hp : 64 * hp + 64, hf, ts(i, 128)],
                                QT[64 * hp : 64 * hp + 64, hf, ds(qs + al, 512 - al)],
                                start=True, stop=True,
                            )
                    return S

                # one flat (h, i) stream with scores emitted one step ahead:
                # PE.SEQ is in-order, so S(next) must be issued before
                # attnV(cur) parks the queue on exp(cur) -- including across
                # head boundaries.
                jobs = [(h, i) for h in range(HLOCc) for i in range(ilast + 1)]
                # splice the previous pass's AG+out-proj emission a quarter and
                # half way into this pass's job stream (by then the AllGather
                # has completed, so the out-proj never parks the in-order
                # PE queue)
                sp1, sp2 = len(jobs) // 4, 17 * len(jobs) // 32
                po_all = {}
                S_next = emit_scores(*jobs[0])
                for idx, (h, i) in enumerate(jobs):
                    hf, hp = h // 2, h % 2
                    if p > 0 and idx in (sp1, sp2):
                        pending_agproj[idx == sp2](p - 1)
                    S = S_next
                    if idx + 1 < len(jobs):
                        S_next = emit_scores(*jobs[idx + 1])
                    if i == 0:
                        po_all[h] = [psop.tile([HDc + 1, 512], F32, tag="otile", name="po")
                                     for _ in range(NSUP)]
                    po = po_all[h]
                    astart = max(0, 128 * i - p * PW)
                    P = pbp.tile([128, PW], BF16, tag="ptile", name="P")
                    pre = (astart // 512) * 512
                    if astart > pre:
                        nc.vector.memset(P[:, ds(pre, astart - pre)], 0.0)
                    nc.scalar.activation(
                        P[:, ds(astart, PW - astart)],
                        S[:, ds(astart, PW - astart)],
                        mybir.ActivationFunctionType.Exp,
                        scale=float(cfg.scale),
                    )
                    if p * PW <= 128 * i:  # diagonal block lives in this pass
                        nc.vector.tensor_mul(P[:, ds(astart, 128)], P[:, ds(astart, 128)], trimask)
                    for j2 in range(NSUP):
                        qs = p * PW + 512 * j2
                        if 128 * i < qs + 512:
                            ilastc = min(ilast, (qs + 512) // 128 - 1)
                            nc.tensor.matmul(
                                po[j2],
                                Vb[:, i, ds((HDc + 1) * h, HDc + 1)],
                                P[:, ts(j2, 512)],
                                start=(i == 0), stop=(i == ilastc),
                            )
                            if i == ilastc:
                                # the copy exists to free the PSUM accumulator
                                # for the next head; the last head of the last
                                # pass normalizes straight from PSUM (shorter
                                # end-of-kernel chain)
                                if p == cfg.NPASS - 1 and h == HLOCc - 1:
                                    osrc = po[j2]
                                else:
                                    osrc = nrm.tile([HDc + 1, 512], F32, tag="osnap", name="osnap")
                                    nc.vector.tensor_copy(osrc, po[j2])
                                rec = nrm.tile([1, 512], F32, tag="rec", name="rec")
                                nc.vector.reciprocal(rec, osrc[HDc : HDc + 1, :])
                                rb = nrm.tile([64, 512], F32, tag="rb", name="rb")
                                nc.gpsimd.partition_broadcast(rb, rec)
                                nc.vector.tensor_mul(
                                    OTs[64 * hp : 64 * hp + 64, hf, ds(p * PW + 512 * j2, 512)],
                                    osrc[0:HDc, :],
                                    rb,
                                )
                # ---- allgather + out-projection per 512-token super (smaller
                # units expose less serial tail after the last attention work).
                # For non-final passes the emission is deferred into the next
                # pass's job stream: PE.SEQ is in-order, and out-proj parked on
                # the AllGather would otherwise stall the next pass's scores.
                def emit_agproj(p, j2):
                    q0 = p * PW + 512 * j2
                    ag_in = dramp.tile([NFT * 128, 512], BF16, tag="agin", name="ag_in")
                    # NOTE: Shared-output collectives need >4 cores/group; with
                    # 4-core groups the output must be a Local scratch tensor.
                    ag_out = dramp.tile([GRP * NFT * 128, 512], BF16, tag="agout", name="ag_out")
                    for t in range(NFT):
                        nc.sync.dma_start(ag_in[ts(t, 128), :], OTs[:, t, ds(q0, 512)])
                    nc.gpsimd.collective_compute(
                        "AllGather",
                        mybir.AluOpType.bypass,
                        ins=[ag_in.opt()],
                        outs=[ag_out.opt()],
                        replica_groups=groups,
                    )
                    OF = ofp.tile([128, NDM, 512], BF16, tag="of", name="OF")
                    # per-chunk loads: the first out-proj matmul starts after
                    # 128KB instead of the full 1MB gathered-feature transfer
                    agv = ag_out.rearrange("(c p) q -> c p q", p=128)
                    for c in range(NDM):
                        nc.sync.dma_start(OF[:, c, :], agv[c])
                    osb = osbp.tile([128, 4, FLOCc], F32, tag="osb", name="osb")
                    outv = out[ds(q0, 512), :].rearrange("(t p) f -> p t f", p=128)
                    for ttl in range(4):
                        pout_full = psout.tile([128, 512], F32, tag="work", name="pout_full")
                        pout = pout_full[:, 0:FLOCc]
                        for c in range(NDM):
                            nc.tensor.matmul(
                                pout, OF[:, c, ts(ttl, 128)], wob[:, c, :],
                                start=(c == 0), stop=False,
                            )
                        nc.tensor.matmul(pout, ones_r, bob, start=False, stop=True)
                        nc.vector.tensor_copy(osb[:, ttl, :], pout)
                        if ttl == 1:
                            nc.sync.dma_start(outv[:, 0:2, :], osb[:, 0:2, :])
                    nc.sync.dma_start(outv[:, 2:4, :], osb[:, 2:4, :])

                if p == cfg.NPASS - 1:
                    for j2 in range(NSUP):
                        emit_agproj(p, j2)
        psum_cm.__exit__(None, None, None)


def make_program(cfg=None, groups=None, unroll=1):
    cfg = cfg or Cfg()
    groups = groups or REPLICA_GROUPS
    nc = bacc.Bacc("TRN2", target_bir_lowering=False, debug=False, num_devices=NCORES)
    x = nc.dram_tensor("x", [cfg.L, cfg.DM], F32, kind="ExternalInput").ap()
    wqkv = nc.dram_tensor("wqkv", [cfg.DM, 3 * cfg.FLOC], F32, kind="ExternalInput").ap()
    bq = nc.dram_tensor("bq", [cfg.FLOC], F32, kind="ExternalInput").ap()
    bk = nc.dram_tensor("bk", [cfg.FLOC], F32, kind="ExternalInput").ap()
    bv = nc.dram_tensor("bv", [cfg.FLOC], F32, kind="ExternalInput").ap()
    wo = nc.dram_tensor("wo", [cfg.DM, cfg.FLOC], F32, kind="ExternalInput").ap()
    bo = nc.dram_tensor("bo", [cfg.FLOC], F32, kind="ExternalInput").ap()
    out = nc.dram_tensor("out", [cfg.L, cfg.FLOC], F32, kind="ExternalOutput").ap()
    with tile.TileContext(nc) as tc:
        nc.tc = tc
        for _ in range(unroll):
            build_body(nc, cfg, x, wqkv, bq, bk, bv, wo, bo, out, groups)
    nc.compile()
    return nc


def shard_inputs(x, w_qkv, b_qkv, w_out, b_out, cfg=None):
    """Full inputs -> list of 8 per-core input dicts."""
    cfg = cfg or Cfg()
    FL = cfg.FLOC
    DMF = cfg.DM
    in_maps = []
    for c in range(NCORES):
        b, r = divmod(c, GRP)
        q0 = r * FL
        in_maps.append({
            "x": np.ascontiguousarray(x[b]),
            "wqkv": np.ascontiguousarray(np.concatenate([
                w_qkv[:, q0 : q0 + FL],
                w_qkv[:, DMF + q0 : DMF + q0 + FL],
                w_qkv[:, 2 * DMF + q0 : 2 * DMF + q0 + FL],
            ], axis=1)),
            "bq": np.ascontiguousarray(b_qkv[q0 : q0 + FL]),
            "bk": np.ascontiguousarray(b_qkv[DMF + q0 : DMF + q0 + FL]),
            "bv": np.ascontiguousarray(b_qkv[2 * DMF + q0 : 2 * DMF + q0 + FL]),
            "wo": np.ascontiguousarray(w_out[:, q0 : q0 + FL]),
            "bo": np.ascontiguousarray(b_out[q0 : q0 + FL]),
        })
    return in_maps


def gather_output(results, cfg=None):
    cfg = cfg or Cfg()
    FL = cfg.FLOC
    out = np.empty((BS, cfg.L, cfg.DM), np.float32)
    for c in range(NCORES):
        b, r = divmod(c, GRP)
        out[b, :, r * FL : (r + 1) * FL] = results[c]["out"]
    return out


_PROGRAM = None


def _get_program():
    global _PROGRAM
    if _PROGRAM is None:
        _PROGRAM = make_program()
    return _PROGRAM


def kernel(x, w_qkv, b_qkv, w_out, b_out):
    x = np.asarray(x, np.float32)
    w_qkv = np.asarray(w_qkv, np.float32)
    b_qkv = np.asarray(b_qkv, np.float32)
    w_out = np.asarray(w_out, np.float32)
    b_out = np.asarray(b_out, np.float32)
    nc = _get_program()
    in_maps = shard_inputs(x, w_qkv, b_qkv, w_out, b_out)
    res = run_bass_kernel_spmd(nc, in_maps, list(range(NCORES)))
    return gather_output(res.results)
